# revision 15
# baseline (speedup 1.0000x reference)
"""DoReFa dense layer (bitW=1, bitA=3) on 8 Trainium2 NeuronCores.

out = quantize_act(clip(|x|,0,1), 3b) @ (sign(W) * mean|W|) + b

Math (exact):
    a_int = round(min(7*|x|, 7))   in {0..7}   -> exact in fp8
    S     = sign(W)                in {-1,0,1} -> exact in fp8
    out   = (E/7) * (a_int @ S) + b,  E = mean|W|

The integer matmul accumulates exactly in fp32 PSUM (|sums| <= 28672 < 2^24).
Both operands are uploaded pre-quantized in fp8 (the quantizers are cheap
elementwise host ops; shipping 3-bit activations as fp8 and 1-bit signs as
fp8 cuts per-core HBM reads from 50.3MB to 21MB), so the device kernel is a
pure fp8 DoubleRow GEMM + scaled eviction. E/7 rides along as a [1,1] input.

Sharding: data-parallel over batch (8 x 1024 rows), S replicated. Activations
land directly in the matmul's stationary DoubleRow layout; sign tiles land in
the moving layout, streamed n-block-major so the PE pipeline starts ~6us in
and runs gapless at the fp8 DoubleRow roofline.
"""

import sys

sys.path.insert(0, "/opt/trn_rl_repo")

from contextlib import ExitStack

import numpy as np
from concourse import bacc, mybir, tile
from concourse.bass_utils import run_bass_kernel_spmd

# Problem dims (hardcoded per contract)
BATCH, IN_CH, N_UNITS = 8192, 4096, 4096
N_CORES = 8
P = 128

M = BATCH // N_CORES  # 1024 rows per core
KO2 = IN_CH // (2 * P)  # 16 DoubleRow k-pair groups of 256
MT = M // P  # 8 m-subtiles of 128
NBS = 512  # n-block width (one PSUM bank)
NB = N_UNITS // NBS  # 8 n-blocks

F32 = mybir.dt.float32
FP8 = mybir.dt.float8e4
AF = mybir.ActivationFunctionType
ALU = mybir.AluOpType
DR = mybir.MatmulPerfMode.DoubleRow


def _body(ctx, tc, a, s, e, b, out, add_bias):
    nc = tc.nc

    outr = out.rearrange("(mt p) n -> mt p n", p=P)

    const = ctx.enter_context(tc.tile_pool(name="const", bufs=1))
    orow_pool = ctx.enter_context(tc.tile_pool(name="orow", bufs=8))
    psum_pool = ctx.enter_context(tc.tile_pool(name="psum", bufs=8, space="PSUM"))

    # Resident tensors: activations 32KB/part, signs 8x16KB/part
    aT = const.tile([P, KO2, 2, M], FP8, name="aT")
    S = [const.tile([P, KO2, 2, NBS], FP8, name=f"S{nb}") for nb in range(NB)]
    sAP = const.tile([P, 1], F32, name="sAP")

    # E/7 arrives host-pre-broadcast as [P,1] — a gpsimd partition_broadcast
    # runs for tens of us on the Q7 core and would block the gpsimd DMA queue.
    # Its DMA is issued with the input stream below.
    if add_bias:
        b_bc = const.tile([P, N_UNITS], F32, name="b_bc")
        nc.scalar.dma_start(b_bc[0:1, :], b[:])
        nc.gpsimd.partition_broadcast(b_bc[:], b_bc[0:1, :], channels=P)

    # PE warm-up: dummy matmuls bridge the framework's ~4.6us startup
    # barrier to the first data arrival (~7.5us) so the HAM clock gate is
    # ramping when the real stream starts. (Tile requires written tiles,
    # hence the memsets.)
    wu_a = const.tile([P, 2, P], FP8, name="wu_a")
    wu_s = const.tile([P, 2, NBS // 2], FP8, name="wu_s")
    nc.vector.memset(wu_a[:], 0.0)
    nc.vector.memset(wu_s[:], 0.0)
    wu_ps = psum_pool.tile([P, NBS // 2], F32, tag="ps", name="wu_ps")
    for _ in range(24):
        nc.tensor.matmul(wu_ps[:], wu_a[:], wu_s[:], start=True, stop=True,
                         perf_mode=DR)

    # Input streaming on two queues (~230-260GB/s each, ~8.1us/10.1us cold
    # start after the framework barrier). The first k-pairs ride the
    # earlier-starting sync queue in fine chunks so the first real matmul
    # fires ~9.7us; block 0's k-pair-outer sweep then chases arrivals. S1 is
    # split across both queues so it completes (~26us) before the m-outer
    # block 1 starts (~37us); later S blocks land 20us+ early. e rides
    # along before the first eviction needs it.
    nc.sync.dma_start(S[0][:, 0:4], s[0][:, 0:4])
    nc.scalar.dma_start(aT[:, 0:2], a[:, 0:2])
    nc.scalar.dma_start(aT[:, 2:4], a[:, 2:4])
    nc.sync.dma_start(aT[:, 4:8], a[:, 4:8])
    nc.scalar.dma_start(S[0][:, 4:8], s[0][:, 4:8])
    nc.sync.dma_start(S[0][:, 8:12], s[0][:, 8:12])
    nc.scalar.dma_start(aT[:, 8:12], a[:, 8:12])
    nc.sync.dma_start(aT[:, 12:16], a[:, 12:16])
    nc.scalar.dma_start(S[0][:, 12:16], s[0][:, 12:16])
    nc.scalar.dma_start(S[1][:, 0:8], s[1][:, 0:8])
    nc.sync.dma_start(S[1][:, 8:16], s[1][:, 8:16])
    nc.scalar.dma_start(sAP[:], e[:])
    for nb in [3, 5]:
        nc.sync.dma_start(S[nb][:], s[nb])
    for nb in [2, 4, 6, 7]:
        nc.scalar.dma_start(S[nb][:], s[nb])

    # Main GEMM: per n-block, 8 psum banks (one per m-subtile). Block 0 runs
    # k-pair-outer to chase the input stream; later blocks run m-outer so each
    # bank's eviction (scale by E/7 on alternating DVE/ACT) spreads across the
    # block instead of bunching at its end. Outputs ride the gpsimd queue
    # early (scalar/sync carry inputs until ~55us) and fan out later.
    for nb in range(NB):
        psums = [
            psum_pool.tile([P, NBS], F32, tag="ps", name=f"ps{nb}_{m}")
            for m in range(MT)
        ]

        def mm(kc, m):
            nc.tensor.matmul(
                psums[m][:],
                aT[:, kc, :, m * P : (m + 1) * P],
                S[nb][:, kc],
                start=(kc == 0),
                stop=(kc == KO2 - 1),
                perf_mode=DR,
            )

        sl = slice(nb * NBS, (nb + 1) * NBS)

        def evict(m):
            ob = orow_pool.tile([P, NBS], F32, tag="orow", name=f"o{nb}_{m}")
            # Last block flips parity so the final eviction is the faster
            # DVE path; elsewhere alternate DVE/ACT to share the load.
            on_dve = (m % 2 == 0) if nb < NB - 1 else (m % 2 == 1)
            if on_dve:
                nc.vector.tensor_scalar(ob[:], psums[m][:], sAP[:], None, ALU.mult)
            else:
                nc.scalar.activation(ob[:], psums[m][:], AF.Copy, scale=sAP[:])
            if add_bias:
                nc.vector.tensor_tensor(ob[:], ob[:], b_bc[:, sl], ALU.add)
            # gpsimd issues DMAs ~0.8us apart on the Q7 core and its queue
            # drains slowly — keep it off the final block's critical tail,
            # and off entirely when the bias broadcast occupies the Q7.
            if add_bias:
                oeng = (nc.sync, nc.scalar)[m % 2]
            elif nb < 4:
                oeng = nc.gpsimd
            elif nb < NB - 1:
                oeng = (nc.gpsimd, nc.sync, nc.scalar)[m % 3]
            else:
                oeng = (nc.sync, nc.scalar)[m % 2]
            oeng.dma_start(outr[m][:, sl], ob[:])

        if nb == 0:
            for kc in range(KO2):
                for m in range(MT):
                    mm(kc, m)
            for m in range(MT):
                evict(m)
        else:
            for m in range(MT):
                for kc in range(KO2):
                    mm(kc, m)
                evict(m)


def build(add_bias=False):
    nc = bacc.Bacc(
        "TRN2", target_bir_lowering=False, debug=False, num_devices=N_CORES
    )
    a = nc.dram_tensor("a", [P, KO2, 2, M], FP8, kind="ExternalInput").ap()
    s = nc.dram_tensor("s", [NB, P, KO2, 2, NBS], FP8, kind="ExternalInput").ap()
    e = nc.dram_tensor("e", [P, 1], F32, kind="ExternalInput").ap()
    b = (
        nc.dram_tensor("b", [1, N_UNITS], F32, kind="ExternalInput").ap()
        if add_bias
        else None
    )
    out = nc.dram_tensor("out", [M, N_UNITS], F32, kind="ExternalOutput").ap()
    with tile.TileContext(nc) as tc, ExitStack() as ctx:
        _body(ctx, tc, a, s, e, b, out, add_bias)
    nc.compile()
    return nc


_cached = {}


def _get_nc(add_bias):
    if add_bias not in _cached:
        _cached[add_bias] = build(add_bias=add_bias)
    return _cached[add_bias]


def _expected_inputs(nc):
    import concourse.mybir as mb

    names = set()
    for alloc in nc.m.functions[0].allocations:
        if isinstance(alloc, mb.MemoryLocationSet) and alloc.kind == "ExternalInput":
            names.add(alloc.memorylocations[0].name)
    return names


def prep_a(x_shard):
    """round(min(7|x|,7)) as fp8, in the DoubleRow stationary layout
    [p, kc, t, m] with k = kc*256 + 2p + t. f32 host math matches the
    reference's f32 round-half-even bit-exactly; 0..7 are exact in fp8."""
    import ml_dtypes

    x = np.asarray(x_shard, dtype=np.float32)
    aq = np.rint(np.minimum(np.abs(x), np.float32(1.0)) * np.float32(7.0))
    a8 = aq.astype(ml_dtypes.float8_e4m3)  # [m, k]
    a8 = a8.reshape(M, KO2, P, 2)
    return np.ascontiguousarray(a8.transpose(2, 1, 3, 0))


def prep_s(W):
    """sign(W) as fp8 (+-1 and 0 are exact), pre-tiled per n-block in the
    DoubleRow moving layout [nb, p, kc, t, n] with k = kc*256 + 2p + t."""
    import ml_dtypes

    s8 = np.sign(np.asarray(W, dtype=np.float32)).astype(ml_dtypes.float8_e4m3)
    s8 = s8.reshape(KO2, P, 2, NB, NBS)
    return np.ascontiguousarray(s8.transpose(3, 1, 0, 2, 4))


def run(inputs, W, b, trace=False):
    add_bias = bool(np.any(b))
    nc = _get_nc(add_bias)
    want = _expected_inputs(nc)
    s8 = prep_s(W)
    e = np.full(
        (P, 1), np.abs(W).mean(dtype=np.float64) / 7.0, dtype=np.float32
    )
    b2 = np.ascontiguousarray(b.reshape(1, -1).astype(np.float32, copy=False))
    in_maps = []
    for c in range(N_CORES):
        a8 = prep_a(inputs[c * M : (c + 1) * M])
        full = {"a": a8, "s": s8, "e": e, "b": b2}
        in_maps.append({k: v for k, v in full.items() if k in want})
    res = run_bass_kernel_spmd(
        nc, in_maps, core_ids=list(range(N_CORES)), trace=trace
    )
    out = np.concatenate([res.results[c]["out"] for c in range(N_CORES)], axis=0)
    return out, res


def kernel(inputs, W, b):
    out, _ = run(inputs, W, b, trace=False)
    return out


if __name__ == "__main__":
    rng = np.random.default_rng(0)
    x = rng.standard_normal((BATCH, IN_CH), dtype=np.float32)
    W = (rng.standard_normal((IN_CH, N_UNITS)) * 0.1).astype(np.float32)
    b = np.zeros(N_UNITS, dtype=np.float32)
    got = kernel(inputs=x, W=W, b=b)
    E = np.abs(W).mean(dtype=np.float64)
    a = np.round(np.minimum(np.abs(x), 1.0) * 7.0)
    want = (a.astype(np.float64) @ np.sign(W).astype(np.float64)) * (E / 7.0)
    err = np.abs(got - want).max() / np.abs(want).max()
    print("rel err vs numpy ref:", err)


# revision 16
# speedup vs baseline: 1.1837x; 1.1837x over previous
"""DoReFa dense layer (bitW=1, bitA=3) on 8 Trainium2 NeuronCores.

out = quantize_act(clip(|x|,0,1), 3b) @ (sign(W) * mean|W|) + b

Math (exact):
    a_int = round(min(7*|x|, 7))   in {0..7}   -> exact in fp8
    S     = sign(W)                in {-1,0,1} -> exact in fp8
    out   = (E/7) * (a_int @ S) + b,  E = mean|W|

The integer matmul accumulates exactly in fp32 PSUM (|sums| <= 28672 < 2^24).
Both operands are uploaded pre-quantized in fp8 (the quantizers are cheap
elementwise host ops; shipping 3-bit activations as fp8 and 1-bit signs as
fp8 cuts per-core HBM reads from 50.3MB to 21MB), so the device kernel is a
pure fp8 DoubleRow GEMM + scaled eviction. E/7 rides along pre-broadcast as
a [128,1] input (a gpsimd partition_broadcast would block its DMA queue).

Sharding: data-parallel over batch (8 x 1024 rows), S replicated.

Schedule (measured on HW): the PE streams 1024 DoubleRow matmuls at
~213ns/512-col each (~218us, the fp8 DR floor — 1 moving column/cycle at
2.4GHz, LdWeights fully hidden). Activations land directly in the stationary
DoubleRow layout, sign tiles in the moving layout, streamed n-block-major on
the two input DMA queues so block 0 chases arrivals (k-pair-outer) from
~12us (framework barrier ~4.6us + DMA queue cold start ~8us) and blocks 1-7
run m-outer, gapless, with evictions spread across each block on alternating
DVE/ACT. Outputs ride the otherwise-idle gpsimd queue early and move to the
sync/scalar queues once those finish the input stream; the last block avoids
gpsimd's slow descriptor issue entirely. Warmup matmuls bridge the startup
barrier so the HAM clock gate is ramped when real work arrives.
"""

import sys

sys.path.insert(0, "/opt/trn_rl_repo")

from contextlib import ExitStack

import numpy as np
from concourse import bacc, mybir, tile
from concourse.bass_utils import run_bass_kernel_spmd

# Problem dims (hardcoded per contract)
BATCH, IN_CH, N_UNITS = 8192, 4096, 4096
N_CORES = 8
P = 128

M = BATCH // N_CORES  # 1024 rows per core
KO2 = IN_CH // (2 * P)  # 16 DoubleRow k-pair groups of 256
MT = M // P  # 8 m-subtiles of 128
NBS = 512  # n-block width (one PSUM bank)
NB = N_UNITS // NBS  # 8 n-blocks

F32 = mybir.dt.float32
FP8 = mybir.dt.float8e4
AF = mybir.ActivationFunctionType
ALU = mybir.AluOpType
DR = mybir.MatmulPerfMode.DoubleRow


def _body(ctx, tc, a, s, e, b, out, add_bias):
    nc = tc.nc

    outr = out.rearrange("(mt p) n -> mt p n", p=P)

    const = ctx.enter_context(tc.tile_pool(name="const", bufs=1))
    orow_pool = ctx.enter_context(tc.tile_pool(name="orow", bufs=8))
    psum_pool = ctx.enter_context(tc.tile_pool(name="psum", bufs=8, space="PSUM"))

    # Resident tensors: activations 32KB/part, signs 8x16KB/part
    aT = const.tile([P, KO2, 2, M], FP8, name="aT")
    S = [const.tile([P, KO2, 2, NBS], FP8, name=f"S{nb}") for nb in range(NB)]
    sAP = const.tile([P, 1], F32, name="sAP")

    # E/7 arrives host-pre-broadcast as [P,1] — a gpsimd partition_broadcast
    # runs for tens of us on the Q7 core and would block the gpsimd DMA queue.
    # Its DMA is issued with the input stream below.
    if add_bias:
        b_bc = const.tile([P, N_UNITS], F32, name="b_bc")
        nc.scalar.dma_start(b_bc[0:1, :], b[:])
        nc.gpsimd.partition_broadcast(b_bc[:], b_bc[0:1, :], channels=P)

    # PE warm-up: dummy matmuls bridge the framework's ~4.6us startup
    # barrier to the first data arrival (~7.5us) so the HAM clock gate is
    # ramping when the real stream starts. (Tile requires written tiles,
    # hence the memsets.)
    wu_a = const.tile([P, 2, P], FP8, name="wu_a")
    wu_s = const.tile([P, 2, NBS // 2], FP8, name="wu_s")
    nc.vector.memset(wu_a[:], 0.0)
    nc.vector.memset(wu_s[:], 0.0)
    wu_ps = psum_pool.tile([P, NBS // 2], F32, tag="ps", name="wu_ps")
    for _ in range(24):
        nc.tensor.matmul(wu_ps[:], wu_a[:], wu_s[:], start=True, stop=True,
                         perf_mode=DR)

    # Input streaming on two queues (~230-260GB/s each, ~8.1us/10.1us cold
    # start after the framework barrier). The first k-pairs ride the
    # earlier-starting sync queue in fine chunks so the first real matmul
    # fires ~9.7us; block 0's k-pair-outer sweep then chases arrivals. S1 is
    # split across both queues so it completes (~26us) before the m-outer
    # block 1 starts (~37us); later S blocks land 20us+ early. e rides
    # along before the first eviction needs it.
    nc.sync.dma_start(S[0][:, 0:4], s[0][:, 0:4])
    nc.scalar.dma_start(aT[:, 0:2], a[:, 0:2])
    nc.scalar.dma_start(aT[:, 2:4], a[:, 2:4])
    nc.sync.dma_start(aT[:, 4:8], a[:, 4:8])
    nc.scalar.dma_start(S[0][:, 4:8], s[0][:, 4:8])
    nc.sync.dma_start(S[0][:, 8:12], s[0][:, 8:12])
    nc.scalar.dma_start(aT[:, 8:12], a[:, 8:12])
    nc.sync.dma_start(aT[:, 12:16], a[:, 12:16])
    nc.scalar.dma_start(S[0][:, 12:16], s[0][:, 12:16])
    nc.scalar.dma_start(S[1][:, 0:8], s[1][:, 0:8])
    nc.sync.dma_start(S[1][:, 8:16], s[1][:, 8:16])
    nc.scalar.dma_start(sAP[:], e[:])
    for nb in [3, 5]:
        nc.sync.dma_start(S[nb][:], s[nb])
    for nb in [2, 4, 6, 7]:
        nc.scalar.dma_start(S[nb][:], s[nb])

    # Main GEMM: per n-block, 8 psum banks (one per m-subtile). Block 0 runs
    # k-pair-outer to chase the input stream; later blocks run m-outer so each
    # bank's eviction (scale by E/7 on alternating DVE/ACT) spreads across the
    # block instead of bunching at its end. Outputs ride the gpsimd queue
    # early (scalar/sync carry inputs until ~55us) and fan out later.
    for nb in range(NB):
        psums = [
            psum_pool.tile([P, NBS], F32, tag="ps", name=f"ps{nb}_{m}")
            for m in range(MT)
        ]

        def mm(kc, m):
            nc.tensor.matmul(
                psums[m][:],
                aT[:, kc, :, m * P : (m + 1) * P],
                S[nb][:, kc],
                start=(kc == 0),
                stop=(kc == KO2 - 1),
                perf_mode=DR,
            )

        sl = slice(nb * NBS, (nb + 1) * NBS)

        def evict(m):
            ob = orow_pool.tile([P, NBS], F32, tag="orow", name=f"o{nb}_{m}")
            # Last block flips parity so the final eviction is the faster
            # DVE path; elsewhere alternate DVE/ACT to share the load.
            on_dve = (m % 2 == 0) if nb < NB - 1 else (m % 2 == 1)
            if on_dve:
                nc.vector.tensor_scalar(ob[:], psums[m][:], sAP[:], None, ALU.mult)
            else:
                nc.scalar.activation(ob[:], psums[m][:], AF.Copy, scale=sAP[:])
            if add_bias:
                nc.vector.tensor_tensor(ob[:], ob[:], b_bc[:, sl], ALU.add)
            # gpsimd issues DMAs ~0.8us apart on the Q7 core and its queue
            # drains slowly — keep it off the final block's critical tail,
            # and off entirely when the bias broadcast occupies the Q7.
            if add_bias:
                oeng = (nc.sync, nc.scalar)[m % 2]
            elif nb < 4:
                oeng = nc.gpsimd
            elif nb < NB - 1:
                oeng = (nc.gpsimd, nc.sync, nc.scalar)[m % 3]
            else:
                oeng = (nc.sync, nc.scalar)[m % 2]
            oeng.dma_start(outr[m][:, sl], ob[:])

        if nb == 0:
            for kc in range(KO2):
                for m in range(MT):
                    mm(kc, m)
            for m in range(MT):
                evict(m)
        else:
            for m in range(MT):
                for kc in range(KO2):
                    mm(kc, m)
                evict(m)


def build(add_bias=False):
    nc = bacc.Bacc(
        "TRN2", target_bir_lowering=False, debug=False, num_devices=N_CORES
    )
    a = nc.dram_tensor("a", [P, KO2, 2, M], FP8, kind="ExternalInput").ap()
    s = nc.dram_tensor("s", [NB, P, KO2, 2, NBS], FP8, kind="ExternalInput").ap()
    e = nc.dram_tensor("e", [P, 1], F32, kind="ExternalInput").ap()
    b = (
        nc.dram_tensor("b", [1, N_UNITS], F32, kind="ExternalInput").ap()
        if add_bias
        else None
    )
    out = nc.dram_tensor("out", [M, N_UNITS], F32, kind="ExternalOutput").ap()
    with tile.TileContext(nc) as tc, ExitStack() as ctx:
        _body(ctx, tc, a, s, e, b, out, add_bias)
    nc.compile()
    return nc


_cached = {}


def _get_nc(add_bias):
    if add_bias not in _cached:
        _cached[add_bias] = build(add_bias=add_bias)
    return _cached[add_bias]


def _expected_inputs(nc):
    import concourse.mybir as mb

    names = set()
    for alloc in nc.m.functions[0].allocations:
        if isinstance(alloc, mb.MemoryLocationSet) and alloc.kind == "ExternalInput":
            names.add(alloc.memorylocations[0].name)
    return names


def prep_a(x_shard):
    """round(min(7|x|,7)) as fp8, in the DoubleRow stationary layout
    [p, kc, t, m] with k = kc*256 + 2p + t. f32 host math matches the
    reference's f32 round-half-even bit-exactly; 0..7 are exact in fp8."""
    import ml_dtypes

    x = np.asarray(x_shard, dtype=np.float32)
    aq = np.rint(np.minimum(np.abs(x), np.float32(1.0)) * np.float32(7.0))
    a8 = aq.astype(ml_dtypes.float8_e4m3)  # [m, k]
    a8 = a8.reshape(M, KO2, P, 2)
    return np.ascontiguousarray(a8.transpose(2, 1, 3, 0))


def prep_s(W):
    """sign(W) as fp8 (+-1 and 0 are exact), pre-tiled per n-block in the
    DoubleRow moving layout [nb, p, kc, t, n] with k = kc*256 + 2p + t."""
    import ml_dtypes

    s8 = np.sign(np.asarray(W, dtype=np.float32)).astype(ml_dtypes.float8_e4m3)
    s8 = s8.reshape(KO2, P, 2, NB, NBS)
    return np.ascontiguousarray(s8.transpose(3, 1, 0, 2, 4))


def run(inputs, W, b, trace=False):
    add_bias = bool(np.any(b))
    nc = _get_nc(add_bias)
    want = _expected_inputs(nc)
    s8 = prep_s(W)
    e = np.full(
        (P, 1), np.abs(W).mean(dtype=np.float64) / 7.0, dtype=np.float32
    )
    b2 = np.ascontiguousarray(b.reshape(1, -1).astype(np.float32, copy=False))
    in_maps = []
    for c in range(N_CORES):
        a8 = prep_a(inputs[c * M : (c + 1) * M])
        full = {"a": a8, "s": s8, "e": e, "b": b2}
        in_maps.append({k: v for k, v in full.items() if k in want})
    res = run_bass_kernel_spmd(
        nc, in_maps, core_ids=list(range(N_CORES)), trace=trace
    )
    out = np.concatenate([res.results[c]["out"] for c in range(N_CORES)], axis=0)
    return out, res


def kernel(inputs, W, b):
    out, _ = run(inputs, W, b, trace=False)
    return out


if __name__ == "__main__":
    rng = np.random.default_rng(0)
    x = rng.standard_normal((BATCH, IN_CH), dtype=np.float32)
    W = (rng.standard_normal((IN_CH, N_UNITS)) * 0.1).astype(np.float32)
    b = np.zeros(N_UNITS, dtype=np.float32)
    got = kernel(inputs=x, W=W, b=b)
    E = np.abs(W).mean(dtype=np.float64)
    a = np.round(np.minimum(np.abs(x), 1.0) * 7.0)
    want = (a.astype(np.float64) @ np.sign(W).astype(np.float64)) * (E / 7.0)
    err = np.abs(got - want).max() / np.abs(want).max()
    print("rel err vs numpy ref:", err)


# revision 19
# speedup vs baseline: 1.1856x; 1.0017x over previous
"""DoReFa dense layer (bitW=1, bitA=3) on 8 Trainium2 NeuronCores.

out = quantize_act(clip(|x|,0,1), 3b) @ (sign(W) * mean|W|) + b

Math (exact):
    a_int = round(min(7*|x|, 7))   in {0..7}   -> exact in fp8
    S     = sign(W)                in {-1,0,1} -> exact in fp8
    out   = (E/7) * (a_int @ S) + b,  E = mean|W|

The integer matmul accumulates exactly in fp32 PSUM (|sums| <= 28672 < 2^24).
Both operands are uploaded pre-quantized in fp8 (the quantizers are cheap
elementwise host ops; shipping 3-bit activations as fp8 and 1-bit signs as
fp8 cuts per-core HBM reads from 50.3MB to 21MB), so the device kernel is a
pure fp8 DoubleRow GEMM + scaled eviction. E/7 rides along pre-broadcast as
a [128,1] input (a gpsimd partition_broadcast would block its DMA queue).

Sharding: data-parallel over batch (8 x 1024 rows), S replicated.

Schedule (measured on HW): the PE streams 1024 DoubleRow matmuls at
~213ns/512-col each (~218us, the fp8 DR floor — 1 moving column/cycle at
2.4GHz, LdWeights fully hidden). Activations land directly in the stationary
DoubleRow layout, sign tiles in the moving layout, streamed n-block-major on
the two input DMA queues so block 0 chases arrivals (k-pair-outer) from
~12us (framework barrier ~4.6us + DMA queue cold start ~8us) and blocks 1-7
run m-outer, gapless, with evictions spread across each block on alternating
DVE/ACT. Outputs ride the otherwise-idle gpsimd queue early and move to the
sync/scalar queues once those finish the input stream; the last block avoids
gpsimd's slow descriptor issue entirely. Warmup matmuls bridge the startup
barrier so the HAM clock gate is ramped when real work arrives.
"""

import sys

sys.path.insert(0, "/opt/trn_rl_repo")

from contextlib import ExitStack

import numpy as np
from concourse import bacc, mybir, tile
from concourse.bass_utils import run_bass_kernel_spmd

# Problem dims (hardcoded per contract)
BATCH, IN_CH, N_UNITS = 8192, 4096, 4096
N_CORES = 8
P = 128

M = BATCH // N_CORES  # 1024 rows per core
KO2 = IN_CH // (2 * P)  # 16 DoubleRow k-pair groups of 256
MT = M // P  # 8 m-subtiles of 128
NBS = 512  # n-block width (one PSUM bank)
NB = N_UNITS // NBS  # 8 n-blocks

F32 = mybir.dt.float32
FP8 = mybir.dt.float8e4
AF = mybir.ActivationFunctionType
ALU = mybir.AluOpType
DR = mybir.MatmulPerfMode.DoubleRow


def _body(ctx, tc, a, s, e, b, out, add_bias):
    nc = tc.nc

    outr = out.rearrange("(mt p) n -> mt p n", p=P)

    const = ctx.enter_context(tc.tile_pool(name="const", bufs=1))
    orow_pool = ctx.enter_context(tc.tile_pool(name="orow", bufs=8))
    psum_pool = ctx.enter_context(tc.tile_pool(name="psum", bufs=8, space="PSUM"))

    # Resident tensors: activations 32KB/part, signs 8x16KB/part
    aT = const.tile([P, KO2, 2, M], FP8, name="aT")
    S = [const.tile([P, KO2, 2, NBS], FP8, name=f"S{nb}") for nb in range(NB)]
    sAP = const.tile([P, 1], F32, name="sAP")

    # E/7 arrives host-pre-broadcast as [P,1] — a gpsimd partition_broadcast
    # runs for tens of us on the Q7 core and would block the gpsimd DMA queue.
    # Its DMA is issued with the input stream below.
    if add_bias:
        b_bc = const.tile([P, N_UNITS], F32, name="b_bc")
        nc.scalar.dma_start(b_bc[0:1, :], b[:])
        nc.gpsimd.partition_broadcast(b_bc[:], b_bc[0:1, :], channels=P)

    # PE warm-up: dummy matmuls bridge the framework's ~4.6us startup
    # barrier to the first data arrival (~7.5us) so the HAM clock gate is
    # ramping when the real stream starts. (Tile requires written tiles,
    # hence the memsets.)
    wu_a = const.tile([P, 2, P], FP8, name="wu_a")
    wu_s = const.tile([P, 2, NBS // 2], FP8, name="wu_s")
    nc.vector.memset(wu_a[:], 0.0)
    nc.vector.memset(wu_s[:], 0.0)
    wu_ps = psum_pool.tile([P, NBS // 2], F32, tag="ps", name="wu_ps")
    for _ in range(24):
        nc.tensor.matmul(wu_ps[:], wu_a[:], wu_s[:], start=True, stop=True,
                         perf_mode=DR)

    # Input streaming on two queues (~230-260GB/s each, ~8.1us/10.1us cold
    # start after the framework barrier). The first k-pairs ride the
    # earlier-starting sync queue in fine chunks so the first real matmul
    # fires ~9.7us; block 0's k-pair-outer sweep then chases arrivals. S1 is
    # split across both queues so it completes (~26us) before the m-outer
    # block 1 starts (~37us); later S blocks land 20us+ early. e rides
    # along before the first eviction needs it.
    nc.sync.dma_start(aT[:, 0:1], a[:, 0:1])
    nc.scalar.dma_start(S[0][:, 0:1], s[0][:, 0:1])
    nc.sync.dma_start(S[0][:, 1:2], s[0][:, 1:2])
    nc.scalar.dma_start(aT[:, 1:2], a[:, 1:2])
    nc.sync.dma_start(aT[:, 2:4], a[:, 2:4])
    nc.scalar.dma_start(S[0][:, 2:4], s[0][:, 2:4])
    nc.scalar.dma_start(S[0][:, 4:8], s[0][:, 4:8])
    nc.sync.dma_start(aT[:, 4:8], a[:, 4:8])
    nc.sync.dma_start(S[0][:, 8:12], s[0][:, 8:12])
    nc.scalar.dma_start(aT[:, 8:12], a[:, 8:12])
    nc.scalar.dma_start(S[0][:, 12:16], s[0][:, 12:16])
    nc.sync.dma_start(aT[:, 12:16], a[:, 12:16])
    nc.scalar.dma_start(S[1][:, 0:8], s[1][:, 0:8])
    nc.sync.dma_start(S[1][:, 8:16], s[1][:, 8:16])
    nc.scalar.dma_start(sAP[:], e[:])
    for nb in [3, 5]:
        nc.sync.dma_start(S[nb][:], s[nb])
    for nb in [2, 4, 6, 7]:
        nc.scalar.dma_start(S[nb][:], s[nb])

    # Main GEMM: per n-block, 8 psum banks (one per m-subtile). Block 0 runs
    # k-pair-outer to chase the input stream; later blocks run m-outer so each
    # bank's eviction (scale by E/7 on alternating DVE/ACT) spreads across the
    # block instead of bunching at its end. Outputs ride the gpsimd queue
    # early (scalar/sync carry inputs until ~55us) and fan out later.
    for nb in range(NB):
        psums = [
            psum_pool.tile([P, NBS], F32, tag="ps", name=f"ps{nb}_{m}")
            for m in range(MT)
        ]

        def mm(kc, m):
            nc.tensor.matmul(
                psums[m][:],
                aT[:, kc, :, m * P : (m + 1) * P],
                S[nb][:, kc],
                start=(kc == 0),
                stop=(kc == KO2 - 1),
                perf_mode=DR,
            )

        sl = slice(nb * NBS, (nb + 1) * NBS)

        def evict(m):
            ob = orow_pool.tile([P, NBS], F32, tag="orow", name=f"o{nb}_{m}")
            # Last block flips parity so the final eviction is the faster
            # DVE path; elsewhere alternate DVE/ACT to share the load.
            on_dve = (m % 2 == 0) if nb < NB - 1 else (m % 2 == 1)
            if on_dve:
                nc.vector.tensor_scalar(ob[:], psums[m][:], sAP[:], None, ALU.mult)
            else:
                nc.scalar.activation(ob[:], psums[m][:], AF.Copy, scale=sAP[:])
            if add_bias:
                nc.vector.tensor_tensor(ob[:], ob[:], b_bc[:, sl], ALU.add)
            # gpsimd issues DMAs ~0.8us apart on the Q7 core and its queue
            # drains slowly — keep it off the final block's critical tail,
            # and off entirely when the bias broadcast occupies the Q7.
            if nb == NB - 1 and m == MT - 1:
                # The kernel's tail waits on the final DMA's completion
                # semaphore (~1.9us latency); halving the last transfer
                # across both fast queues shaves the data time off it.
                h = NBS // 2
                nc.sync.dma_start(outr[m][:, nb * NBS : nb * NBS + h], ob[:, :h])
                nc.scalar.dma_start(outr[m][:, nb * NBS + h : (nb + 1) * NBS], ob[:, h:])
                return
            if add_bias:
                oeng = (nc.sync, nc.scalar)[m % 2]
            elif nb < 4:
                oeng = nc.gpsimd
            elif nb < NB - 1:
                oeng = (nc.gpsimd, nc.sync, nc.scalar)[m % 3]
            else:
                oeng = (nc.sync, nc.scalar)[m % 2]
            oeng.dma_start(outr[m][:, sl], ob[:])

        if nb == 0:
            # m=7 is deferred to an m-outer chain at the block's end: banks
            # m0-6 then finish (and evict) ~3.4us before the block ends, so
            # block 1's m-outer chains never wait on a psum recycle.
            for kc in range(KO2):
                for m in range(MT - 1):
                    mm(kc, m)
            for kc in range(KO2):
                mm(kc, MT - 1)
            for m in range(MT):
                evict(m)
        else:
            for m in range(MT):
                for kc in range(KO2):
                    mm(kc, m)
                evict(m)


def build(add_bias=False):
    nc = bacc.Bacc(
        "TRN2", target_bir_lowering=False, debug=False, num_devices=N_CORES
    )
    a = nc.dram_tensor("a", [P, KO2, 2, M], FP8, kind="ExternalInput").ap()
    s = nc.dram_tensor("s", [NB, P, KO2, 2, NBS], FP8, kind="ExternalInput").ap()
    e = nc.dram_tensor("e", [P, 1], F32, kind="ExternalInput").ap()
    b = (
        nc.dram_tensor("b", [1, N_UNITS], F32, kind="ExternalInput").ap()
        if add_bias
        else None
    )
    out = nc.dram_tensor("out", [M, N_UNITS], F32, kind="ExternalOutput").ap()
    with tile.TileContext(nc) as tc, ExitStack() as ctx:
        _body(ctx, tc, a, s, e, b, out, add_bias)
    nc.compile()
    return nc


_cached = {}


def _get_nc(add_bias):
    if add_bias not in _cached:
        _cached[add_bias] = build(add_bias=add_bias)
    return _cached[add_bias]


def _expected_inputs(nc):
    import concourse.mybir as mb

    names = set()
    for alloc in nc.m.functions[0].allocations:
        if isinstance(alloc, mb.MemoryLocationSet) and alloc.kind == "ExternalInput":
            names.add(alloc.memorylocations[0].name)
    return names


def prep_a(x_shard):
    """round(min(7|x|,7)) as fp8, in the DoubleRow stationary layout
    [p, kc, t, m] with k = kc*256 + 2p + t. f32 host math matches the
    reference's f32 round-half-even bit-exactly; 0..7 are exact in fp8."""
    import ml_dtypes

    x = np.asarray(x_shard, dtype=np.float32)
    aq = np.rint(np.minimum(np.abs(x), np.float32(1.0)) * np.float32(7.0))
    a8 = aq.astype(ml_dtypes.float8_e4m3)  # [m, k]
    a8 = a8.reshape(M, KO2, P, 2)
    return np.ascontiguousarray(a8.transpose(2, 1, 3, 0))


def prep_s(W):
    """sign(W) as fp8 (+-1 and 0 are exact), pre-tiled per n-block in the
    DoubleRow moving layout [nb, p, kc, t, n] with k = kc*256 + 2p + t."""
    import ml_dtypes

    s8 = np.sign(np.asarray(W, dtype=np.float32)).astype(ml_dtypes.float8_e4m3)
    s8 = s8.reshape(KO2, P, 2, NB, NBS)
    return np.ascontiguousarray(s8.transpose(3, 1, 0, 2, 4))


def run(inputs, W, b, trace=False):
    add_bias = bool(np.any(b))
    nc = _get_nc(add_bias)
    want = _expected_inputs(nc)
    s8 = prep_s(W)
    e = np.full(
        (P, 1), np.abs(W).mean(dtype=np.float64) / 7.0, dtype=np.float32
    )
    b2 = np.ascontiguousarray(b.reshape(1, -1).astype(np.float32, copy=False))
    in_maps = []
    for c in range(N_CORES):
        a8 = prep_a(inputs[c * M : (c + 1) * M])
        full = {"a": a8, "s": s8, "e": e, "b": b2}
        in_maps.append({k: v for k, v in full.items() if k in want})
    res = run_bass_kernel_spmd(
        nc, in_maps, core_ids=list(range(N_CORES)), trace=trace
    )
    out = np.concatenate([res.results[c]["out"] for c in range(N_CORES)], axis=0)
    return out, res


def kernel(inputs, W, b):
    out, _ = run(inputs, W, b, trace=False)
    return out


if __name__ == "__main__":
    rng = np.random.default_rng(0)
    x = rng.standard_normal((BATCH, IN_CH), dtype=np.float32)
    W = (rng.standard_normal((IN_CH, N_UNITS)) * 0.1).astype(np.float32)
    b = np.zeros(N_UNITS, dtype=np.float32)
    got = kernel(inputs=x, W=W, b=b)
    E = np.abs(W).mean(dtype=np.float64)
    a = np.round(np.minimum(np.abs(x), 1.0) * 7.0)
    want = (a.astype(np.float64) @ np.sign(W).astype(np.float64)) * (E / 7.0)
    err = np.abs(got - want).max() / np.abs(want).max()
    print("rel err vs numpy ref:", err)
